# revision 1
# baseline (speedup 1.0000x reference)
"""DLRM inference kernel for 8 Trainium2 NeuronCores.

Strategy: pure data parallelism. The batch (16384) is split into 8 slices
of 2048; the 26 embedding tables and all MLP weights are replicated to
every core, so no collectives are needed. Eval-mode BatchNorm is folded
into the MLP weights on the host. All on-chip matmuls run feature-major
(contraction dim on partitions); the pairwise-interaction terms are
computed per 128-sample tile with a shifted-overlap trick (one multiply +
one segmented reduce per feature distance) split across the vector engine
and GPSIMD, and the gathered embeddings are transposed into the
feature-major top-MLP input with PE transposes.
"""

import sys

for _p in ("/opt/trn_rl_repo",):
    if _p not in sys.path:
        sys.path.insert(0, _p)

import numpy as np

import bass_rust
import concourse.bass as bass
import concourse.mybir as mybir
import concourse.tile as tile
from concourse.masks import make_identity

# Problem constants (hardcoded per spec nn_DLRM_5403068858958)
B, CD, NF, V, D = 16384, 13, 26, 100000, 64
NCORES = 8
BC = B // NCORES          # 2048 samples per core
BN_INV = 1.0 / np.sqrt(1.0 + 1e-5)
P = 128
NE = NF + 1               # 27 concatenated features (bottom + 26 embeddings)
NEP = NE + 1              # padded to 28 features -> 1792 flat rows
FLAT_ROWS = NEP * D       # 1792 (14 k-tiles)
NPAIR = NE * (NE - 1) // 2  # 351
INTER_ROWS = 384          # 351 padded to 3 k-tiles
K_TILES = (FLAT_ROWS + INTER_ROWS) // P  # 17
TOPK = FLAT_ROWS + INTER_ROWS            # 2176
F32 = mybir.dt.float32
I32 = mybir.dt.int32
HB = 256                  # samples per SBUF-resident chunk
GP_DELTAS = (1, 2, 3, 4)  # interaction multiplies offloaded to GPSIMD


def _split_multiwaits(nc):
    """The walrus build here accepts at most ONE sync wait per instruction.
    Hoist extra waits onto single-wait NoOps inserted immediately before the
    carrying instruction on the same engine (always sound: the waits only
    move earlier by zero instructions)."""
    n_extra = 0
    for fn in nc.m.functions:
        for blk in fn.blocks:
            insts = blk.instructions
            out = []
            for inst in insts:
                si = inst.sync_info
                waits = list(si.on_wait) if si is not None else []
                if len(waits) > 1:
                    for k, w in enumerate(waits[:-1]):
                        nop = bass_rust.InstNoOp(name=f"{inst.name}-sw{k}")
                        nop.engine = inst.engine
                        nop.bass_nofuse = True
                        nop.sync_info = bass_rust.SyncInfo(
                            on_wait=[w], on_update=[])
                        nc.register_instruction(nop, overwrite=True)
                        out.append(nop)
                        n_extra += 1
                    inst.sync_info = bass_rust.SyncInfo(
                        on_wait=[waits[-1]], on_update=list(si.on_update))
                out.append(inst)
            blk.instructions = out
    return n_extra


def _pair_maps():
    """Map my interaction row order (by distance delta, then i) to the
    reference np.triu row-major order, as an index array my_of_ref."""
    ref = {}
    k = 0
    for i in range(NE):
        for j in range(i + 1, NE):
            ref[(i, j)] = k
            k += 1
    mine = []
    for delta in range(1, NE):
        for i in range(NE - delta):
            mine.append(ref[(i, i + delta)])
    return np.array(mine, dtype=np.int64)  # mine[m] = ref index of my row m


def build_nc(b_core=BC, hb=HB, loop_n=1, ablate=()):
    """Build the per-core Bass kernel. b_core samples, processed in chunks
    of hb (hb must divide b_core; both multiples of 128). loop_n > 1 wraps
    the whole computation in a hardware loop (for timing)."""
    assert b_core % hb == 0 and hb % P == 0
    n_chunk = b_core // hb
    tiles_per_chunk = hb // P

    nc = bass.Bass()
    # ---- DRAM I/O ----
    xT = nc.dram_tensor("xT", [CD, b_core], F32, kind="ExternalInput")
    idx = nc.dram_tensor("idx", [b_core, NF], I32, kind="ExternalInput")
    tabs = nc.dram_tensor("tabs", [NF * V, D], F32, kind="ExternalInput")
    w1 = nc.dram_tensor("w1", [CD, 256], F32, kind="ExternalInput")
    b1 = nc.dram_tensor("b1", [P, 2], F32, kind="ExternalInput")
    w2 = nc.dram_tensor("w2", [256, P], F32, kind="ExternalInput")
    b2 = nc.dram_tensor("b2", [P, 1], F32, kind="ExternalInput")
    w3 = nc.dram_tensor("w3", [P, D], F32, kind="ExternalInput")
    b3 = nc.dram_tensor("b3", [D, 1], F32, kind="ExternalInput")
    w4 = nc.dram_tensor("w4", [TOPK, 512], F32, kind="ExternalInput")
    b4 = nc.dram_tensor("b4", [P, 4], F32, kind="ExternalInput")
    w5 = nc.dram_tensor("w5", [512, 256], F32, kind="ExternalInput")
    b5 = nc.dram_tensor("b5", [P, 2], F32, kind="ExternalInput")
    w6 = nc.dram_tensor("w6", [256, P], F32, kind="ExternalInput")
    b6 = nc.dram_tensor("b6", [P, 1], F32, kind="ExternalInput")
    w7 = nc.dram_tensor("w7", [P, 1], F32, kind="ExternalInput")
    b7 = nc.dram_tensor("b7", [1, 1], F32, kind="ExternalInput")
    scT = nc.dram_tensor("scT", [1, b_core], F32, kind="ExternalOutput")

    with tile.TileContext(nc) as tc:
        with (
            tc.tile_pool(name="const", bufs=1) as constp,
            tc.tile_pool(name="fm", bufs=2) as fmp,
            tc.tile_pool(name="acts", bufs=2) as actp,
            tc.tile_pool(name="g", bufs=4) as gp,
            tc.tile_pool(name="pr", bufs=2) as prp,
            tc.tile_pool(name="prg", bufs=3) as prgp,
            tc.tile_pool(name="outp", bufs=2) as outp,
            tc.tile_pool(name="mmps", bufs=4, space="PSUM") as mmps,
            tc.tile_pool(name="tps", bufs=3, space="PSUM") as tps,
        ):
            # ---- all gather indices in one early DMA ----
            n_tiles_all0 = b_core // P
            idxall = constp.tile([P, n_tiles_all0, NF], I32)
            nc.sync.dma_start(
                idxall[:], idx[:].rearrange("(t p) f -> p t f", p=P))

            # ---- persistent constants / weights in SBUF ----
            ident = constp.tile([P, P], F32)
            make_identity(nc, ident[:])
            w1s = constp.tile([CD, 256], F32)
            nc.sync.dma_start(w1s[:], w1[:])
            b1s = constp.tile([P, 2], F32)
            nc.sync.dma_start(b1s[:], b1[:])
            w2s = constp.tile([P, 2, P], F32)
            nc.sync.dma_start(w2s[:], w2[:].rearrange("(c p) m -> p c m", p=P))
            b2s = constp.tile([P, 1], F32)
            nc.sync.dma_start(b2s[:], b2[:])
            w3s = constp.tile([P, D], F32)
            nc.sync.dma_start(w3s[:], w3[:])
            b3s = constp.tile([D, 1], F32)
            nc.sync.dma_start(b3s[:], b3[:])
            w4s = constp.tile([P, K_TILES, 512], F32)
            nc.sync.dma_start(w4s[:], w4[:].rearrange("(c p) m -> p c m", p=P))
            b4s = constp.tile([P, 4], F32)
            nc.sync.dma_start(b4s[:], b4[:])
            w5s = constp.tile([P, 4, 256], F32)
            nc.sync.dma_start(w5s[:], w5[:].rearrange("(c p) m -> p c m", p=P))
            b5s = constp.tile([P, 2], F32)
            nc.sync.dma_start(b5s[:], b5[:])
            w6s = constp.tile([P, 2, P], F32)
            nc.sync.dma_start(w6s[:], w6[:].rearrange("(c p) m -> p c m", p=P))
            b6s = constp.tile([P, 1], F32)
            nc.sync.dma_start(b6s[:], b6[:])
            w7s = constp.tile([P, 1], F32)
            nc.sync.dma_start(w7s[:], w7[:])
            b7s = constp.tile([1, 1], F32)
            nc.sync.dma_start(b7s[:], b7[:])

            n_tiles_all = b_core // P

            def emit_body(iv=None):
                # ---------- bottom MLP for the whole core batch ----------
                xTs = actp.tile([CD, b_core], F32, tag="xTs", bufs=1)
                nc.sync.dma_start(xTs[:], xT[:])
                h1T = actp.tile([P, 2, b_core], F32, tag="h1T", bufs=1)
                h2T = actp.tile([P, b_core], F32, tag="h2T", bufs=1)
                bT = actp.tile([D, b_core], F32, tag="bT", bufs=1)
                bsm = actp.tile([P, n_tiles_all, D], F32, tag="bsm", bufs=1)
                BW = min(512, b_core)
                for nck in range(b_core // BW):
                    nsl = slice(nck * BW, (nck + 1) * BW)
                    for mc in range(2):
                        ps = mmps.tile([P, BW], F32, tag="mmps")
                        nc.tensor.matmul(
                            ps[:], w1s[:, mc * P:(mc + 1) * P], xTs[:, nsl],
                            start=True, stop=True)
                        nc.scalar.activation(
                            h1T[:, mc, nsl], ps[:],
                            mybir.ActivationFunctionType.Relu,
                            bias=b1s[:, mc:mc + 1])
                    ps = mmps.tile([P, BW], F32, tag="mmps")
                    for kc in range(2):
                        nc.tensor.matmul(
                            ps[:], w2s[:, kc, :], h1T[:, kc, nsl],
                            start=(kc == 0), stop=(kc == 1))
                    nc.scalar.activation(
                        h2T[:, nsl], ps[:],
                        mybir.ActivationFunctionType.Relu, bias=b2s[:, 0:1])
                    ps = mmps.tile([P, BW], F32, tag="mmps")
                    nc.tensor.matmul(
                        ps[:D], w3s[:], h2T[:, nsl], start=True, stop=True)
                    nc.vector.tensor_add(
                        bT[:, nsl], ps[:D], b3s[:].to_broadcast([D, BW]))
                for tg in range(n_tiles_all):
                    tp = tps.tile([P, P], F32, tag="tp")
                    nc.tensor.transpose(
                        tp[:, :D], bT[:, tg * P:(tg + 1) * P], ident[:D, :D])
                    nc.any.tensor_copy(bsm[:, tg, :], tp[:, :D])

                for h in range(n_chunk):
                    xs = slice(h * hb, (h + 1) * hb)
                    fm = fmp.tile([P, K_TILES, hb], F32, tag="fm")
                    t1T = actp.tile([P, 4, hb], F32, tag="t1T")
                    t2T = actp.tile([P, 2, hb], F32, tag="t2T")
                    t3T = actp.tile([P, hb], F32, tag="t3T")
                    # ---------- gather phase for the chunk ----------
                    allgs = []
                    for t in range(tiles_per_chunk):
                        tg0 = h * tiles_per_chunk + t
                        it = idxall[:, tg0, :]
                        allg = gp.tile([P, NEP, D], F32, tag="allg")
                        for f in (range(0) if 'gather' in ablate
                                  else range(NF)):
                            nc.gpsimd.indirect_dma_start(
                                out=allg[:, f + 1, :], out_offset=None,
                                in_=tabs[:],
                                in_offset=bass.IndirectOffsetOnAxis(
                                    ap=it[:, f:f + 1], axis=0))
                        tg = h * tiles_per_chunk + t
                        nc.any.tensor_copy(allg[:, 0, :], bsm[:, tg, :])
                        nc.any.memset(allg[:, NE, :], 0.0)
                        gpprs = {}
                        for delta in GP_DELTAS:
                            n = NE - delta
                            pr = prgp.tile([P, NF, D], F32, tag="prg")
                            nc.gpsimd.tensor_tensor(
                                pr[:, :n, :], allg[:, 0:n, :],
                                allg[:, delta:delta + n, :],
                                op=mybir.AluOpType.mult)
                            gpprs[delta] = pr
                        allgs.append((allg, gpprs))

                    # ---------- per 128-sample tile ----------
                    for t in range(tiles_per_chunk):
                        col = slice(t * P, (t + 1) * P)
                        allg, gpprs = allgs[t]
                        # ---------- interactions ----------
                        intt = prp.tile([P, INTER_ROWS], F32, tag="intt")
                        nc.any.memset(intt[:, NPAIR:], 0.0)
                        deltas = [] if 'inter' in ablate else (
                            [d for d in range(1, NE) if d not in GP_DELTAS]
                            + [d for d in GP_DELTAS])
                        # GP-computed deltas go last so DVE isn't head-blocked
                        # waiting on GPSIMD at tile start.
                        offs = np.concatenate(
                            [[0], np.cumsum([NE - d for d in range(1, NE)])])
                        for delta in deltas:
                            n = NE - delta
                            off = int(offs[delta - 1])
                            if delta in GP_DELTAS:
                                pr = gpprs[delta]
                            else:
                                pr = prp.tile([P, NF, D], F32, tag="pr")
                                nc.vector.tensor_tensor(
                                    pr[:, :n, :],
                                    allg[:, 0:n, :],
                                    allg[:, delta:delta + n, :],
                                    op=mybir.AluOpType.mult)
                            nc.vector.tensor_reduce(
                                intt[:, off:off + n], pr[:, :n, :],
                                axis=mybir.AxisListType.X,
                                op=mybir.AluOpType.add)

                        # ---------- transposes into feature-major fm ----------
                        for c in range(0) if 'tr' in ablate else range(NEP // 2):
                            tp = tps.tile([P, P], F32, tag="tp")
                            nc.tensor.transpose(
                                tp[:], allg[:, 2 * c:2 * c + 2, :], ident[:])
                            nc.any.tensor_copy(fm[:, c, col], tp[:])
                        for j in range(INTER_ROWS // P):
                            tp = tps.tile([P, P], F32, tag="tp")
                            nc.tensor.transpose(
                                tp[:], intt[:, j * P:(j + 1) * P], ident[:])
                            nc.any.tensor_copy(fm[:, NEP // 2 + j, col], tp[:])

                    # ---------- top MLP (feature-major) ----------
                    for mc in range(4):
                        ps = mmps.tile([P, hb], F32, tag="mmps")
                        for kc in range(K_TILES):
                            nc.tensor.matmul(
                                ps[:], w4s[:, kc, mc * P:(mc + 1) * P],
                                fm[:, kc, :],
                                start=(kc == 0), stop=(kc == K_TILES - 1))
                        nc.scalar.activation(
                            t1T[:, mc, :], ps[:],
                            mybir.ActivationFunctionType.Relu,
                            bias=b4s[:, mc:mc + 1])
                    for mc in range(2):
                        ps = mmps.tile([P, hb], F32, tag="mmps")
                        for kc in range(4):
                            nc.tensor.matmul(
                                ps[:], w5s[:, kc, mc * P:(mc + 1) * P],
                                t1T[:, kc, :],
                                start=(kc == 0), stop=(kc == 3))
                        nc.scalar.activation(
                            t2T[:, mc, :], ps[:],
                            mybir.ActivationFunctionType.Relu,
                            bias=b5s[:, mc:mc + 1])
                    ps = mmps.tile([P, hb], F32, tag="mmps")
                    for kc in range(2):
                        nc.tensor.matmul(
                            ps[:], w6s[:, kc, :], t2T[:, kc, :],
                            start=(kc == 0), stop=(kc == 1))
                    nc.scalar.activation(
                        t3T[:], ps[:],
                        mybir.ActivationFunctionType.Relu, bias=b6s[:, 0:1])
                    ps7 = mmps.tile([P, hb], F32, tag="mmps")
                    nc.tensor.matmul(
                        ps7[:1], w7s[:], t3T[:], start=True, stop=True)
                    so = outp.tile([1, hb], F32, tag="so")
                    nc.vector.tensor_add(
                        so[:], ps7[:1], b7s[:].to_broadcast([1, hb]))
                    nc.sync.dma_start(scT[:, xs], so[:])

            for _rep in range(loop_n):
                emit_body()

    _split_multiwaits(nc)
    return nc


def prep_host(inputs, b_core=BC):
    """Fold BN, reorder W4, build per-core input maps."""
    f = lambda a: np.ascontiguousarray(np.asarray(a), dtype=np.float32)
    continuous = f(inputs["continuous"])
    cat_idx = np.asarray(inputs["cat_idx"])
    tabs = f(inputs["emb_tables"]).reshape(NF * V, D)

    s1 = f(inputs["g1"]) * BN_INV
    w1f = f(inputs["W1"]) * s1[None, :]
    b1f = (f(inputs["b1"]) * s1 + f(inputs["be1"])).reshape(2, P).T.copy()
    s2 = f(inputs["g2"]) * BN_INV
    w2f = f(inputs["W2"]) * s2[None, :]
    b2f = (f(inputs["b2"]) * s2 + f(inputs["be2"])).reshape(1, P).T.copy()
    w3f = f(inputs["W3"])
    b3f = f(inputs["b3"]).reshape(D, 1)

    s4 = f(inputs["g4"]) * BN_INV
    W4 = f(inputs["W4"]) * s4[None, :]
    b4f = (f(inputs["b4"]) * s4 + f(inputs["be4"])).reshape(4, P).T.copy()
    my_of_ref = _pair_maps()
    W4m = np.zeros((TOPK, 512), dtype=np.float32)
    W4m[:NE * D] = W4[NPAIR:NPAIR + NE * D]            # flat part (27 features)
    W4m[FLAT_ROWS + np.arange(NPAIR)] = W4[my_of_ref]  # interactions
    s5 = f(inputs["g5"]) * BN_INV
    w5f = f(inputs["W5"]) * s5[None, :]
    b5f = (f(inputs["b5"]) * s5 + f(inputs["be5"])).reshape(2, P).T.copy()
    s6 = f(inputs["g6"]) * BN_INV
    w6f = f(inputs["W6"]) * s6[None, :]
    b6f = (f(inputs["b6"]) * s6 + f(inputs["be6"])).reshape(1, P).T.copy()
    w7f = f(inputs["W7"])
    b7f = f(inputs["b7"]).reshape(1, 1)

    foffs = (np.arange(NF, dtype=np.int64) * V).astype(np.int32)
    in_maps = []
    ncores = B // b_core
    for c in range(ncores):
        sl = slice(c * b_core, (c + 1) * b_core)
        in_maps.append(dict(
            xT=np.ascontiguousarray(continuous[sl].T),
            idx=np.ascontiguousarray(cat_idx[sl].astype(np.int32)
                                     + foffs[None, :]),
            tabs=tabs,
            w1=w1f, b1=b1f, w2=w2f, b2=b2f, w3=w3f, b3=b3f,
            w4=W4m, b4=b4f, w5=w5f, b5=b5f, w6=w6f, b6=b6f,
            w7=w7f, b7=b7f,
        ))
    return in_maps


_NC_CACHE = {}


def kernel(**inputs) -> np.ndarray:
    from concourse.bass_utils import run_bass_kernel_spmd

    key = (BC, HB)
    if key not in _NC_CACHE:
        _NC_CACHE[key] = build_nc(*key)
    nc = _NC_CACHE[key]
    in_maps = prep_host(inputs, BC)
    res = run_bass_kernel_spmd(nc, in_maps, core_ids=list(range(NCORES)))
    out = np.concatenate(
        [r["scT"].reshape(BC, 1) for r in res.results], axis=0)
    return out.astype(np.float32)

